# revision 47
# baseline (speedup 1.0000x reference)
"""Multi-head causal self-attention (N=4, L=2048, E=1024, H=16) on 8 NeuronCores.

Sharding: core c handles batch b = c//2 and head-group g = c%2 (8 heads,
E-slice of 512). Each core computes its QKV projection slice, causal
attention for its 8 heads, and a partial out-projection (E-contraction over
its 512-slice). Host sums the two partials per batch (bias added on g=0).

Key structure (v3, rewritten from the 318us v2):
  - PE warmup burst at t=0 (dummy matmuls) trips the HAM clock gate to
    K=8/8 before the projection stream begins (v2 ran its first 24.6us
    at 1.2 GHz).
  - Scores per (kt, qj) tile: ROW-TILED CONCURRENT pairs (K=64, rows
    0-63 / 64-127) -> both heads of a pair in one N-cycle slot.
  - AV per pair: COL-TILED CONCURRENT pair (M=64 weights, psum cols
    0-63 / 64-127) -> both heads in one N-cycle slot into ONE psum bank
    that directly matches the at2 [es, q] layout (no partition-shift
    DMA anymore).
  - Softmax denominators via M=1 ones-matmuls, four concurrent per slot
    (col groups 0/32/64/96), accumulated in one psum bank per sweep.
  - Attention runs as (jq, tt) sweeps: tt picks two head pairs, so per
    key block the PE does 5 slots (2 score + 2 AV + 1 den) against 4
    exp's on ACT -- balanced, with proj/outproj work interleaved as
    deadline-scheduled PE filler.
  - Reciprocal broadcast via one-hot K=128 matmuls (fp32r col-tiling and
    K<128 col-tiling are rejected by walrus; DVE partition shifts must
    be 32-aligned or the device faults).
  - kt/qj/p2/vts are bf16; exp output bf16; mask-mul on gpsimd.
"""

from collections import deque
from contextlib import ExitStack

import ml_dtypes
import numpy as np

import concourse.bacc as bacc
import concourse.mybir as mybir
import concourse.tile as tile
from concourse import bass_utils

F32 = mybir.dt.float32
F32R = mybir.dt.float32r
BF16 = mybir.dt.bfloat16
AF = mybir.ActivationFunctionType

N, L, E = 4, 2048, 1024
H, EH = 16, 64
NCORES = 8
ES = 512          # e-slice per core (8 heads x 64)
SCALE = 1.0 / np.sqrt(EH)

_CACHE = {}


def _build():
    nc = bacc.Bacc("TRN2", target_bir_lowering=False, debug=False,
                   num_devices=NCORES)
    xq = nc.dram_tensor("xq", (E, L), BF16, kind="ExternalInput").ap()
    xk = nc.dram_tensor("xk", (E, L), BF16, kind="ExternalInput").ap()
    xv = nc.dram_tensor("xv", (E, L), BF16, kind="ExternalInput").ap()
    wq = nc.dram_tensor("wq", (E, ES), BF16, kind="ExternalInput").ap()
    wk = nc.dram_tensor("wk", (E, ES), BF16, kind="ExternalInput").ap()
    wv = nc.dram_tensor("wv", (E, ES), BF16, kind="ExternalInput").ap()
    wo = nc.dram_tensor("wo", (ES, E), BF16, kind="ExternalInput").ap()
    bq = nc.dram_tensor("bq", (128, 4), F32, kind="ExternalInput").ap()
    bk = nc.dram_tensor("bk", (128, 4), F32, kind="ExternalInput").ap()
    bv = nc.dram_tensor("bv", (1, ES), F32, kind="ExternalInput").ap()
    bo = nc.dram_tensor("bo", (1, E), F32, kind="ExternalInput").ap()
    msk = nc.dram_tensor("msk", (128, 128), F32, kind="ExternalInput").ap()
    y = nc.dram_tensor("y", (L, E), F32, kind="ExternalOutput").ap()

    with tile.TileContext(nc) as tc:
        with ExitStack() as stk:
            ec = stk.enter_context
            cpool = ec(tc.tile_pool(name="const", bufs=1))
            ktpool = ec(tc.tile_pool(name="kt", bufs=4))
            vpool = ec(tc.tile_pool(name="vp", bufs=16))
            wkpool = ec(tc.tile_pool(name="wtk", bufs=8))
            wvpool = ec(tc.tile_pool(name="wtv", bufs=8))
            wqpool = ec(tc.tile_pool(name="wtq", bufs=8))
            wopool = ec(tc.tile_pool(name="wo", bufs=4))
            xpool = ec(tc.tile_pool(name="xs", bufs=2))
            qjpool = ec(tc.tile_pool(name="qj", bufs=8))
            a2pool = ec(tc.tile_pool(name="a2", bufs=16))
            ppool = ec(tc.tile_pool(name="pp", bufs=6))
            rpool = ec(tc.tile_pool(name="rp", bufs=2))
            rbpool = ec(tc.tile_pool(name="rb", bufs=2))
            otpool = ec(tc.tile_pool(name="ot", bufs=4))
            # PSUM: sp(score ring 2x2 banks) + avp(2) + dnp(1) + fp(1) = 8
            sp = ec(tc.tile_pool(name="sp", bufs=2, space="PSUM"))
            avp = ec(tc.tile_pool(name="avp", bufs=2, space="PSUM"))
            dnp = ec(tc.tile_pool(name="dnp", bufs=1, space="PSUM"))
            fp = ec(tc.tile_pool(name="fp", bufs=1, space="PSUM"))

            # ---------------- DMA issue: weights & consts ----------------
            # sync queue: wk, xk0, xv0, xq0 (critical path to first MMs)
            # gpsimd queue: small consts, wv, wq, wo
            bq_sb = cpool.tile([128, 4], F32)
            bk_sb = cpool.tile([128, 4], F32)
            bv_sb = cpool.tile([1, ES], F32R)
            bo_sb = cpool.tile([1, E], F32R)
            mtri_f = cpool.tile([128, 128], F32)
            nc.gpsimd.dma_start(out=bq_sb, in_=bq)
            nc.gpsimd.dma_start(out=bk_sb, in_=bk)
            nc.gpsimd.dma_start(out=bv_sb, in_=bv.bitcast(F32R))
            nc.gpsimd.dma_start(out=bo_sb, in_=bo.bitcast(F32R))
            nc.gpsimd.dma_start(out=mtri_f, in_=msk)

            # ---------------- PE warmup burst ----------------------------
            # ~24 dummy matmuls (~4-5us at the cold 1.2 GHz clock) so the
            # HAM activity window sees a busy PE and releases the clock
            # gate before the first kproj matmuls arrive.
            wtmp = cpool.tile([128, 512], BF16)
            nc.vector.memset(wtmp, 0.0)
            for wi in range(24):
                pw = fp.tile([128, 512], F32, tag="fp", name=f"warm{wi}")
                nc.tensor.matmul(pw, wtmp[:, 0:128], wtmp,
                                 start=True, stop=True)

            def load_w(pool, w_dram, nm, eng, split=False):
                ts = []
                for ko in range(8):
                    t = pool.tile([128, ES], BF16, tag=f"w{nm}",
                                  name=f"w{nm}{ko}")
                    if split and ko % 2 == 1:
                        eng = nc.gpsimd
                    elif split:
                        eng = nc.sync
                    eng.dma_start(
                        out=t,
                        in_=w_dram[ko * 128:(ko + 1) * 128, :])
                    ts.append(t)
                return ts

            wk_t = load_w(wkpool, wk, "k", nc.sync, split=True)
            wv_t = load_w(wvpool, wv, "v", nc.gpsimd)
            wq_t = load_w(wqpool, wq, "q", nc.gpsimd)
            wo_t = []
            for pr in range(4):
                t = wopool.tile([128, E], BF16, tag="wo", name=f"wo{pr}")
                nc.gpsimd.dma_start(
                    out=t, in_=wo[pr * 128:(pr + 1) * 128, :])
                wo_t.append(t)

            # ---------------- persistent on-chip tensors -----------------
            kt = [ktpool.tile([128, L], BF16, tag="kt", name=f"kt{i}")
                  for i in range(4)]
            vts = [vpool.tile([128, 8, 64], BF16, tag="v", name=f"v{i}")
                   for i in range(16)]

            ones_st = cpool.tile([1, 128], F32)
            nc.vector.memset(ones_st, 1.0)
            ones = cpool.tile([1, 128], F32R)
            nc.vector.tensor_copy(ones, ones_st)
            # bf16 ones column: den matmul weights (K=128, M=1)
            ones_bf = cpool.tile([128, 1], BF16)
            nc.vector.memset(ones_bf, 1.0)
            # one-hot K=128 M=128 bcast weights: oh[:, pi, :] broadcasts
            # rcr row 64pi to out partitions 0-63 and row 64pi+32 to
            # partitions 64-127 in a single matmul. (walrus rejects
            # col-tiled K<128 or fp32 matmuls, so K=1 ones are out.)
            oh_st = cpool.tile([128, 2, 128], F32)
            nc.vector.memset(oh_st, 0.0)
            for pi in range(2):
                nc.vector.memset(oh_st[64 * pi:64 * pi + 1, pi, 0:64], 1.0)
                nc.vector.memset(
                    oh_st[64 * pi + 32:64 * pi + 33, pi, 64:128], 1.0)
            oh = cpool.tile([128, 2, 128], F32R)
            nc.vector.tensor_copy(oh, oh_st)
            # persistent zero-padded reciprocal rows (rows 0/32/64/96
            # carry data, the rest benign so the one-hot matmul reads no
            # garbage); two buffers to decouple consecutive sweeps
            rcz_st = cpool.tile([128, 512], F32)
            nc.vector.memset(rcz_st, 0.0)
            rcr_ab = []
            for ri in range(2):
                rcrt = cpool.tile([128, 512], F32R, tag=f"rcr{ri}",
                                  name=f"rcr{ri}")
                nc.vector.tensor_copy(rcrt, rcz_st)
                rcr_ab.append(rcrt)
            mtri = cpool.tile([128, 2, 128], BF16)
            nc.vector.tensor_copy(mtri[:, 0, :], mtri_f)
            nc.vector.tensor_copy(mtri[:, 1, :], mtri_f)

            # broadcast bias tiles (via ones-matmul, one-time)
            bv_bc = cpool.tile([128, ES], F32)
            bo_bc = cpool.tile([128, E], F32)

            def make_bcasts():
                ps = fp.tile([128, 512], F32, tag="fp", name="psb0")
                nc.tensor.matmul(ps, ones, bv_sb, start=True, stop=True)
                nc.vector.tensor_copy(bv_bc, ps)
                for half in range(2):
                    ps2 = fp.tile([128, 512], F32, tag="fp", name="psb1")
                    nc.tensor.matmul(ps2, ones,
                                     bo_sb[:, half * 512:(half + 1) * 512],
                                     start=True, stop=True)
                    nc.vector.tensor_copy(
                        bo_bc[:, half * 512:(half + 1) * 512], ps2)

            # ---------------- projection chunk emitters ------------------
            x_cache = {}

            def x_tile(x_dram, lb, key, eng=None):
                # NOTE: xpool ring has bufs=2; allocation order must ensure
                # the slot being reused already has its readers emitted.
                if (key, lb) in x_cache:
                    return x_cache[(key, lb)]
                t = xpool.tile([128, 8, 512], BF16, tag="x", name="xt")
                (eng or nc.sync).dma_start(
                    out=t,
                    in_=x_dram.rearrange("(ko ki) l -> ki ko l", ki=128)
                    [:, :, lb * 512:(lb + 1) * 512])
                x_cache[(key, lb)] = t
                return t

            def kproj_chunk(lb, ep):
                # kt[eo][:, lb*512:+512] for eo in {2ep, 2ep+1}
                def emit():
                    xt = x_tile(xk, lb, "k")
                    ps = sp.tile([128, 2, 512], F32, tag="sp", name="psk")
                    for ei in range(2):
                        eo = 2 * ep + ei
                        for ko in range(8):
                            nc.tensor.matmul(
                                ps[:, ei, :],
                                wk_t[ko][:, eo * 128:(eo + 1) * 128],
                                xt[:, ko, :],
                                start=(ko == 0), stop=(ko == 7))
                    for ei in range(2):
                        eo = 2 * ep + ei
                        nc.vector.tensor_scalar_add(
                            kt[eo][:, lb * 512:(lb + 1) * 512],
                            ps[:, ei, :], bk_sb[:, eo:eo + 1])
                return emit

            def vproj_chunk(lb, pair):
                # vts[lb*4 + {2pair, 2pair+1}]
                def emit():
                    xt = x_tile(xv, lb, "v")
                    ps = sp.tile([128, 2, 512], F32, tag="sp", name="psv")
                    for ii in range(2):
                        i = 2 * pair + ii
                        for ko in range(8):
                            nc.tensor.matmul(
                                ps[:, ii, :],
                                xt[:, ko, i * 128:(i + 1) * 128],
                                wv_t[ko], start=(ko == 0), stop=(ko == 7))
                    for ii in range(2):
                        i = 2 * pair + ii
                        lv = lb * 4 + i
                        nc.vector.tensor_add(
                            vts[lv],
                            ps[:, ii, :].rearrange("p (h e) -> p h e", e=64),
                            bv_bc.rearrange("p (h e) -> p h e", e=64))
                return emit

            qj = {}

            def qproj_chunk(jq, ep):
                # qj[(jq, eo)] for eo in {2ep, 2ep+1}
                def emit():
                    xt = x_tile(xq, jq, "q")
                    ps = sp.tile([128, 2, 512], F32, tag="sp", name="psq")
                    for ei in range(2):
                        eo = 2 * ep + ei
                        for ko in range(8):
                            nc.tensor.matmul(
                                ps[:, ei, :],
                                wq_t[ko][:, eo * 128:(eo + 1) * 128],
                                xt[:, ko, :],
                                start=(ko == 0), stop=(ko == 7))
                    for ei in range(2):
                        eo = 2 * ep + ei
                        t = qjpool.tile([128, 512], BF16, tag="qj",
                                        name=f"qj{jq}_{eo}")
                        nc.vector.tensor_scalar_add(
                            t, ps[:, ei, :], bq_sb[:, eo:eo + 1])
                        qj[(jq, eo)] = t
                return emit

            at2 = {}

            def outproj_part(jq, lc, no):
                def emit():
                    a = at2[jq]
                    psf = fp.tile([128, 512], F32, tag="fp", name="psf")
                    for pr in range(4):
                        nc.tensor.matmul(
                            psf,
                            a[pr][:, lc * 128:(lc + 1) * 128],
                            wo_t[pr][:, no * 512:(no + 1) * 512],
                            start=(pr == 0), stop=(pr == 3))
                    ot = otpool.tile([128, 512], F32, tag="ot", name="ot")
                    nc.vector.tensor_add(
                        ot, psf, bo_bc[:, no * 512:(no + 1) * 512])
                    nc.sync.dma_start(
                        out=y[jq * 512 + lc * 128:jq * 512 + (lc + 1) * 128,
                              no * 512:(no + 1) * 512],
                        in_=ot)
                return emit

            # ---------------- attention sweep machinery ------------------
            pending = deque()

            def flush_pending(depth):
                while len(pending) > depth:
                    pending.popleft()()

            def mk_av_pair(av, t, kb, n0, nkb, p2):
                # col-tiled concurrent pair: head 2t -> psum rows 0-63,
                # head 2t+1 -> rows 64-127 (tile_position (0,0)/(0,64))
                def emit():
                    nc.tensor.matmul(
                        av[0:64, n0:512], vts[kb][:, 2 * t, :],
                        p2[:, 0, n0:512],
                        start=(kb == 0), stop=(kb == nkb - 1))
                    nc.tensor.matmul(
                        av[64:128, n0:512], vts[kb][:, 2 * t + 1, :],
                        p2[:, 1, n0:512],
                        start=(kb == 0), stop=(kb == nkb - 1))
                return emit

            def mk_den(dnt, kb, n0, nkb, p2a, p2b):
                # 4 concurrent M=1 ones-matmuls (col groups 0/32/64/96):
                # den rows: 0 = head 2t0, 32 = 2t0+1, 64 = 2t1, 96 = 2t1+1
                def emit():
                    for ci, (p2, hh) in enumerate(
                            ((p2a, 0), (p2a, 1), (p2b, 0), (p2b, 1))):
                        nc.tensor.matmul(
                            dnt[32 * ci:32 * ci + 1, n0:512], ones_bf,
                            p2[:, hh, n0:512],
                            start=(kb == 0), stop=(kb == nkb - 1),
                            tile_position=(0, 32 * ci))
                return emit

            norm_count = [0]

            def mk_norm(av0, av1, dnt, jq, t0, t1):
                # reciprocal of denominators, broadcast over partitions via
                # one-hot K=128 matmuls, then normalize psum -> at2 tiles.
                # reciprocal_approx_fast mis-reads large values straight
                # from PSUM (negative garbage); stage through SBUF first
                def emit():
                    rcr = rcr_ab[norm_count[0] % 2]
                    norm_count[0] += 1
                    # den rows live at partitions 0/32/64/96. DVE partition
                    # shifts must be 32-aligned (unaligned shifts fault the
                    # device), so stage at the same rows, fill the gaps
                    # with 1.0 (benign for recip), and run recip/cast on
                    # the whole 97-row block at base partition 0.
                    dsb = rpool.tile([97, 512], F32, tag="dsb", name="dsb")
                    rcf = rpool.tile([97, 512], F32, tag="rcf", name="rcf")
                    nc.vector.memset(dsb, 1.0)
                    for ci in range(4):
                        r = slice(32 * ci, 32 * ci + 1)
                        nc.vector.tensor_copy(dsb[r, :], dnt[r, :])
                    nc.vector.reciprocal_approx_fast(rcf, dsb)
                    nc.vector.tensor_copy(rcr[0:97, :], rcf)
                    # bcast: pair t0 -> dnp bank (reuse), pair t1 -> fp
                    for pi, (av, t) in enumerate(((av0, t0), (av1, t1))):
                        pool = dnp if pi == 0 else fp
                        rb_ps = pool.tile([128, 512], F32,
                                          tag=("dn" if pi == 0 else "fp"),
                                          name=f"rbps{pi}")
                        nc.tensor.matmul(rb_ps, oh[:, pi, :], rcr,
                                         start=True, stop=True)
                        rb = rbpool.tile([128, 512], F32R, tag="rb",
                                         name=f"rb{pi}")
                        nc.vector.tensor_copy(rb, rb_ps)
                        nc.vector.tensor_mul(at2[jq][t], av, rb)
                return emit

            def sweep(jq, tt, fills):
                # process head pairs t0 = 2tt, t1 = 2tt+1 for query block
                # jq: per key block kb, 5 PE slots (2 score, 2 AV, 1 den)
                t0, t1 = 2 * tt, 2 * tt + 1
                nkb = 4 * (jq + 1)
                av0 = avp.tile([128, 512], F32, tag="av", name=f"av0_{jq}{tt}")
                av1 = avp.tile([128, 512], F32, tag="av", name=f"av1_{jq}{tt}")
                dnt = dnp.tile([128, 512], F32, tag="dn", name=f"dn_{jq}{tt}")
                done_fill = 0
                for kb in range(nkb):
                    m = kb - 4 * jq
                    n0 = 128 * m if m >= 0 else 0
                    p2s = []
                    for ti, t in ((0, t0), (1, t1)):
                        pss = sp.tile([128, 2, 512], F32, tag="sp",
                                      name="pss")
                        nc.tensor.matmul(
                            pss[:, 0, n0:512],
                            kt[t][0:64, kb * 128:(kb + 1) * 128],
                            qj[(jq, t)][0:64, n0:512],
                            start=True, stop=True)
                        nc.tensor.matmul(
                            pss[:, 1, n0:512],
                            kt[t][64:128, kb * 128:(kb + 1) * 128],
                            qj[(jq, t)][64:128, n0:512],
                            start=True, stop=True)
                        if ti == 0:
                            flush_pending(5)
                        p2 = ppool.tile([128, 2, 512], BF16, tag="p",
                                        name="p2")
                        nc.scalar.activation(p2[:, :, n0:512],
                                             pss[:, :, n0:512],
                                             AF.Exp, scale=float(SCALE))
                        if m >= 0:
                            # all-SBUF operands: idle gpsimd engine
                            nc.gpsimd.tensor_mul(
                                p2[:, :, n0:n0 + 128],
                                p2[:, :, n0:n0 + 128], mtri)
                        p2s.append(p2)
                    pending.append(mk_av_pair(av0, t0, kb, n0, nkb, p2s[0]))
                    pending.append(mk_av_pair(av1, t1, kb, n0, nkb, p2s[1]))
                    pending.append(mk_den(dnt, kb, n0, nkb, p2s[0], p2s[1]))
                    want = ((kb + 1) * len(fills)) // nkb
                    while done_fill < want:
                        pending.append(fills[done_fill])
                        done_fill += 1
                pending.append(mk_norm(av0, av1, dnt, jq, t0, t1))

            # ---------------- opening (phase A head) ---------------------
            # x ring (bufs=2) allocation order: xk0(A), xv0(B), xq0(A after
            # kproj emitted), then per-seg xk/xv prefetch + mid-seg xq kick.
            x_tile(xk, 0, "k")
            x_tile(xv, 0, "v", eng=nc.scalar)
            kproj_chunk(0, 0)()
            make_bcasts()
            kproj_chunk(0, 1)()
            x_tile(xq, 0, "q")
            vproj_chunk(0, 0)()
            vproj_chunk(0, 1)()
            qproj_chunk(0, 0)()
            qproj_chunk(0, 1)()

            def xq_kick(jq):
                # issue next xq DMA once kproj fillers (readers of the x
                # ring slot being reused) have been emitted
                def emit():
                    x_tile(xq, jq, "q")
                return emit

            def pre(lb):
                def em():
                    x_tile(xk, lb, "k")
                    x_tile(xv, lb, "v")
                return em

            for jq in range(4):
                at2[jq] = [a2pool.tile([128, 512], BF16, tag="a2",
                                       name=f"a2_{jq}_{i}")
                           for i in range(4)]

            op0 = [outproj_part(0, lc, no) for lc in range(4)
                   for no in range(2)]
            op1 = [outproj_part(1, lc, no) for lc in range(4)
                   for no in range(2)]
            op2 = [outproj_part(2, lc, no) for lc in range(4)
                   for no in range(2)]

            # ---------------- main attention stream ----------------------
            schedule = [
                (0, 0, [pre(1), kproj_chunk(1, 0)]),
                (0, 1, [kproj_chunk(1, 1), xq_kick(1), vproj_chunk(1, 0),
                        vproj_chunk(1, 1), qproj_chunk(1, 0),
                        qproj_chunk(1, 1)]),
                (1, 0, [pre(2), kproj_chunk(2, 0), kproj_chunk(2, 1)]),
                (1, 1, [xq_kick(2), vproj_chunk(2, 0), vproj_chunk(2, 1),
                        qproj_chunk(2, 0), qproj_chunk(2, 1)]),
                (2, 0, [pre(3), kproj_chunk(3, 0), kproj_chunk(3, 1),
                        op0[0], op0[1], op0[2], op0[3]]),
                (2, 1, [xq_kick(3), vproj_chunk(3, 0), vproj_chunk(3, 1),
                        qproj_chunk(3, 0), qproj_chunk(3, 1),
                        op0[4], op0[5], op0[6], op0[7]]),
                (3, 0, op1[0:8] + op2[0:4]),
                (3, 1, op2[4:8]),
            ]
            for jq, tt, fills in schedule:
                if tt == 0:
                    # force pending (incl. this jq's qproj fillers) to have
                    # emitted before the first sweep references qj[(jq, t)]
                    flush_pending(0)
                sweep(jq, tt, fills)

            def outproj_tail_pair(pairi):
                # final out-projection: sp pool (idle at the tail) provides
                # two psum banks per pair so parts pipeline instead of
                # serializing on the single fp bank
                def emit():
                    a = at2[3]
                    psf2 = sp.tile([128, 2, 512], F32, tag="sp",
                                   name="psf2")
                    for half in range(2):
                        lc, no = pairi, half
                        for pr in range(4):
                            nc.tensor.matmul(
                                psf2[:, half, :],
                                a[pr][:, lc * 128:(lc + 1) * 128],
                                wo_t[pr][:, no * 512:(no + 1) * 512],
                                start=(pr == 0), stop=(pr == 3))
                    for half in range(2):
                        lc, no = pairi, half
                        ot = otpool.tile([128, 512], F32, tag="ot",
                                         name="ot")
                        nc.vector.tensor_add(
                            ot, psf2[:, half, :],
                            bo_bc[:, no * 512:(no + 1) * 512])
                        nc.sync.dma_start(
                            out=y[3 * 512 + lc * 128:
                                  3 * 512 + (lc + 1) * 128,
                                  no * 512:(no + 1) * 512],
                            in_=ot)
                return emit

            for pairi in range(4):
                pending.append(outproj_tail_pair(pairi))
            flush_pending(0)

    nc.finalize()
    return nc


def _make_tri():
    kk = np.arange(128)[:, None]
    jj = np.arange(128)[None, :]
    return (jj >= kk).astype(np.float32)


def make_in_maps(query, key, value, W_packed, b_packed, W_out, b_out):
    query = np.asarray(query, dtype=np.float32)
    key = np.asarray(key, dtype=np.float32)
    value = np.asarray(value, dtype=np.float32)
    W_packed = np.asarray(W_packed, dtype=np.float32)
    b_packed = np.asarray(b_packed, dtype=np.float32)
    W_out = np.asarray(W_out, dtype=np.float32)
    b_out = np.asarray(b_out, dtype=np.float32)

    msk = _make_tri()
    BF = ml_dtypes.bfloat16
    xqT = [np.ascontiguousarray(query[b].T).astype(BF) for b in range(N)]
    xkT = [np.ascontiguousarray(key[b].T).astype(BF) for b in range(N)]
    xvT = [np.ascontiguousarray(value[b].T).astype(BF) for b in range(N)]

    in_maps = []
    for c in range(NCORES):
        b, g = c // 2, c % 2
        sl = slice(g * ES, (g + 1) * ES)
        in_maps.append({
            "xq": xqT[b], "xk": xkT[b], "xv": xvT[b],
            "wq": np.ascontiguousarray(
                W_packed[0 * E:][:E][sl, :].T).astype(BF),
            "wk": np.ascontiguousarray(
                W_packed[1 * E:][:E][sl, :].T).astype(BF),
            "wv": np.ascontiguousarray(
                W_packed[2 * E:][:E][sl, :].T).astype(BF),
            "wo": np.ascontiguousarray(W_out[:, sl].T).astype(BF),
            "bq": np.ascontiguousarray(
                b_packed[0 * E:][:E][sl].reshape(4, 128).T),
            "bk": np.ascontiguousarray(
                b_packed[1 * E:][:E][sl].reshape(4, 128).T),
            "bv": b_packed[2 * E:][:E][sl].reshape(1, ES).copy(),
            "bo": (b_out.reshape(1, E).copy() if g == 0
                   else np.zeros((1, E), np.float32)),
            "msk": msk,
        })
    return in_maps


def get_nc():
    if "nc" not in _CACHE:
        _CACHE["nc"] = _build()
    return _CACHE["nc"]


def kernel(query, key, value, W_packed, b_packed, W_out, b_out):
    nc = get_nc()
    in_maps = make_in_maps(query, key, value, W_packed, b_packed,
                           W_out, b_out)
    res = bass_utils.run_bass_kernel_spmd(nc, in_maps,
                                          core_ids=list(range(NCORES)))
    out = np.stack([res.results[2 * b]["y"] + res.results[2 * b + 1]["y"]
                    for b in range(N)])
    return out.astype(np.float32)
